# revision 19
# baseline (speedup 1.0000x reference)
"""CRF negative log-likelihood on 8 NeuronCores, sharded over TIME.

The 511-step forward recurrence q_t = e_t * (E^T q_{t-1}) is split into 8
chunks of 64 steps.  Each core carries, for every batch element, the full
48x48 transfer matrix of its chunk (identity-initialized), so all chunks
run concurrently: one blockdiag-bf16 matmul plus a broadcast multiply by
diag(e_t) per step, batched over (2 batches x 48 partitions, 48 basis x 32
batch-pairs columns).  Emissions are host-normalized (exact logsumexp + a
constant drift term) so no renormalization is ever needed.  Per-sequence
end states S_j (lengths are host-known) and the final chunk matrices are
DMA'd out; the host stitches chunk boundaries and computes the gold-path
score in float64.
"""
import os
import sys
import numpy as np

sys.path.insert(0, "/opt/trn_rl_repo")

import ml_dtypes
import concourse.bass as bass
import concourse.bacc as bacc
import concourse.mybir as mybir
import concourse.tile as tile
from concourse.bass_utils import run_bass_kernel_spmd

T, B, L = 512, 64, 48
START, STOP = 46, 47
NCORES = 8
K = 64                      # scan steps per core (core 7: 63 real + 1 pad)
MU = 0.4287                 # mean per-step log-growth, keeps |log q| < ~5
P2 = 2 * L                  # 96 partitions: (batch parity, label)
NB2 = B // 2                # 32 batch pairs
SC = L * NB2                # 1536 state columns: (basis j, batch pair b2)

_FP = mybir.dt.float32
_BF = mybir.dt.bfloat16
_cache = {}


def _build(endings):
    """endings: per-core tuple of (slot, b2, k) extraction points."""
    nc = bacc.Bacc()
    dT = nc.declare_dram_parameter("dT", [P2, K * NB2], _FP, isOutput=False)
    w2 = nc.declare_dram_parameter("w2", [P2, P2], _BF, isOutput=False)
    sinit = nc.declare_dram_parameter("sinit", [P2, SC], _BF, isOutput=False)
    mout = nc.declare_dram_parameter("mout", [P2, SC], _BF, isOutput=True)
    sel = nc.declare_dram_parameter("sel", [P2, B * L], _BF, isOutput=True)

    by_k = [[] for _ in range(K)]
    for (slot, b2, k) in endings:
        by_k[k].append((slot, b2))

    with tile.TileContext(nc) as tc:
        with (
            tc.tile_pool(name="consts", bufs=1) as consts,
            tc.tile_pool(name="state", bufs=1) as state,
            tc.tile_pool(name="spool", bufs=3) as spool,
            tc.tile_pool(name="tpool", bufs=3) as tpool,
            tc.tile_pool(name="ps0", bufs=2, space="PSUM") as ps0,
            tc.tile_pool(name="ps1", bufs=2, space="PSUM") as ps1,
            tc.tile_pool(name="ps2", bufs=2, space="PSUM") as ps2,
        ):
            w2_sb = consts.tile([P2, P2], _BF)
            nc.gpsimd.dma_start(w2_sb[:], w2[:])
            s0 = state.tile([P2, SC], _BF)
            nc.gpsimd.dma_start(s0[:], sinit[:])
            dT_sb = state.tile([P2, K * NB2], _FP)
            for c in range(2):
                cs = slice(c * K * NB2 // 2, (c + 1) * K * NB2 // 2)
                nc.gpsimd.dma_start(dT_sb[:, cs], dT[:, cs])
            dh = state.tile([P2, K * NB2], _BF)
            for c in range(4):
                cs = slice(c * K * NB2 // 4, (c + 1) * K * NB2 // 4)
                nc.scalar.activation(dh[:, cs], dT_sb[:, cs],
                                     mybir.ActivationFunctionType.Exp)

            s_prev = s0
            for k in range(K):
                s_new = spool.tile([P2, SC], _BF, tag="s")
                tmp = tpool.tile([P2, 1024], _BF, tag="tmp")
                dsl = dh[:, k * NB2:(k + 1) * NB2]
                p0 = ps0.tile([P2, 512], _FP, tag="p0")
                nc.tensor.matmul(p0[:], w2_sb[:], s_prev[:, 0:512])
                nc.scalar.copy(tmp[:, 0:512], p0[:])
                p1 = ps1.tile([P2, 512], _FP, tag="p1")
                nc.tensor.matmul(p1[:], w2_sb[:], s_prev[:, 512:1024])
                nc.scalar.copy(tmp[:, 512:1024], p1[:])
                p2 = ps2.tile([P2, 512], _FP, tag="p2")
                nc.tensor.matmul(p2[:], w2_sb[:], s_prev[:, 1024:1536])
                # D-multiply: cols 0:1024 read the bf16 SBUF bounce (4x DVE
                # mode), cols 1024:1536 read PSUM directly (1x).
                for h in range(2):
                    hs = slice(h * 512, (h + 1) * 512)
                    nc.vector.tensor_mul(
                        s_new[:, hs].rearrange('u (j b) -> u j b', j=16),
                        tmp[:, hs].rearrange('u (j b) -> u j b', j=16),
                        dsl.unsqueeze(1).broadcast_to([P2, 16, NB2]))
                nc.vector.tensor_mul(
                    s_new[:, 1024:1536].rearrange('u (j b) -> u j b', j=16),
                    p2[:].rearrange('u (j b) -> u j b', j=16),
                    dsl.unsqueeze(1).broadcast_to([P2, 16, NB2]))
                for (slot, b2) in by_k[k]:
                    src = s_new[:].rearrange('u (j b) -> u j b', j=L)
                    nc.gpsimd.dma_start(
                        sel[:, slot * L:(slot + 1) * L],
                        src[:, :, b2:b2 + 1].squeeze(2))
                s_prev = s_new
            nc.gpsimd.dma_start(mout[:], s_prev[:])
    nc.finalize()
    return nc


def _get_nc(endings_key, endings):
    if _cache.get("key") != endings_key:
        _cache["nc"] = _build(endings)
        _cache["key"] = endings_key
    return _cache["nc"]


def kernel(feats, transitions, tags, mask):
    feats = np.asarray(feats, np.float32)
    transitions = np.asarray(transitions, np.float32)
    tags_in = np.asarray(tags).astype(np.int64)
    mask_in = np.asarray(mask).astype(bool)

    tr64 = transitions.astype(np.float64)
    E = np.exp(tr64)                       # E[i,j] = exp(trans i->j)
    estop = np.exp(tr64[:, STOP])
    lengths = mask_in.sum(1).astype(np.int64)

    f64 = feats.astype(np.float64)
    m = f64.max(2)
    c = m + np.log(np.exp(f64 - m[:, :, None]).sum(2)) + MU      # (T, B)
    fhat = f64 - c[:, :, None]                                   # (T, B, L)
    Slog = np.cumsum(c, axis=0)
    q0 = np.exp(fhat[0]) * np.exp(tr64[START])[None, :]          # (B, L)

    # per-sequence end step -> (core, slot, b, local k)
    endings = [[] for _ in range(NCORES)]
    host_only = []                    # sequences ending at t=0 (never occurs
    for b in range(B):                # for the spec'd lengths, but be safe)
        tstar = int(lengths[b]) - 1
        if tstar == 0:
            host_only.append(b)
            continue
        cstar = (tstar - 1) // K
        kstar = tstar - K * cstar - 1
        endings[cstar].append((len(endings[cstar]), b, kstar))
    endings_key = tuple(lengths.tolist())
    # build program uses (slot, b2, k) per core; same program for all cores
    # would diverge, so the union of all cores' extraction points runs on
    # every core (harmless extra DMAs), with per-core slot collisions
    # avoided by a global slot numbering.
    all_pts = []
    for ci in range(NCORES):
        for (slot, b, kst) in endings[ci]:
            all_pts.append((ci, slot, b, kst))
    # global slots: one per sequence
    prog_pts = tuple((gslot, b // 2, kst)
                     for gslot, (ci, slot, b, kst) in enumerate(all_pts))
    nc = _get_nc(endings_key, prog_pts)

    w2 = np.zeros((P2, P2), np.float64)
    w2[0:L, 0:L] = E
    w2[L:P2, L:P2] = E
    w2_bf = w2.astype(ml_dtypes.bfloat16)
    sinit = np.zeros((P2, SC), np.float64)
    i_idx = np.arange(L)
    for p in range(2):
        sinit[p * L + i_idx[:, None], i_idx[:, None] * NB2 + np.arange(NB2)[None, :]] = 1.0
    sinit_bf = sinit.astype(ml_dtypes.bfloat16)

    in_maps = []
    for ci in range(NCORES):
        chunk = np.zeros((K, B, L), np.float64)
        lo = K * ci + 1
        hi = min(lo + K, T)
        chunk[:hi - lo] = fhat[lo:hi]
        # dT[p*48+i, k*32+b2] = chunk[k, 2*b2+p, i]
        dTc = np.ascontiguousarray(
            chunk.reshape(K, NB2, 2, L).transpose(2, 3, 0, 1).reshape(
                P2, K * NB2)).astype(np.float32)
        in_maps.append({
            "dT": dTc,
            "w2": w2_bf,
            "sinit": sinit_bf,
        })

    tmpbase = os.environ.get("BASS_KERNEL_TMPDIR")
    if tmpbase:
        import tempfile
        tmpbase = tempfile.mkdtemp(dir=tmpbase)
    bkr = run_bass_kernel_spmd(
        nc, in_maps, list(range(NCORES)), tmpdir=tmpbase)
    global LAST_EXEC_NS
    LAST_EXEC_NS = bkr.exec_time_ns
    res = bkr.results

    # ---- host combine: chunk boundary vectors (float64) ----
    qin = np.zeros((NCORES, B, L))
    qin[0] = q0
    for ci in range(NCORES - 1):
        M = np.asarray(res[ci]["mout"]).astype(np.float64).reshape(
            2, L, L, NB2)                     # (p, i', j, b2)
        qv = qin[ci].reshape(NB2, 2, L)       # (b2, p, j)
        qin[ci + 1] = np.einsum('pijb,bpj->bpi', M, qv).reshape(B, L)

    # ---- forward score from extracted end states ----
    fwd = 0.0
    for gslot, (ci, slot, b, kst) in enumerate(all_pts):
        selc = np.asarray(res[ci]["sel"]).astype(np.float64)
        Sb = selc[:, gslot * L:(gslot + 1) * L]       # (96, 48)
        p = b % 2
        Sjb = Sb[p * L:(p + 1) * L, :]                # (i', j)
        z = estop @ (Sjb @ qin[ci][b])
        tstar = int(lengths[b]) - 1
        fwd += np.log(z) + Slog[tstar, b]
    for b in host_only:
        fwd += np.log(estop @ q0[b]) + Slog[0, b]

    # ---- gold path score (pure gathers, float64) ----
    tagsT = tags_in.T
    prev = np.concatenate([np.full((1, B), START, np.int64), tagsT[:-1]], 0)
    emit = np.take_along_axis(f64, tagsT[:, :, None], 2)[..., 0]
    tg = np.where(mask_in.T, emit + tr64[prev, tagsT], 0.0).sum()
    end_ids = tagsT[lengths - 1, np.arange(B)]
    gold = tg + tr64[end_ids, STOP].sum()

    return np.float32(fwd - gold)


# revision 22
# speedup vs baseline: 18.4136x; 18.4136x over previous
"""CRF negative log-likelihood on 8 NeuronCores, sharded over TIME.

The 511-step forward recurrence q_t = e_t * (E^T q_{t-1}) is split into 8
chunks of 64 steps.  Each core carries, for every batch element, the full
48x48 transfer matrix of its chunk (identity-initialized), so all chunks
run concurrently: one blockdiag-bf16 matmul plus a broadcast multiply by
diag(e_t) per step, batched over (2 batches x 48 partitions, 48 basis x 32
batch-pairs columns).  Emissions are host-normalized (exact logsumexp + a
constant drift term) so no renormalization is ever needed.  Per-sequence
end states S_j (lengths are host-known) and the final chunk matrices are
DMA'd out; the host stitches chunk boundaries and computes the gold-path
score in float64.
"""
import os
import sys
import numpy as np

sys.path.insert(0, "/opt/trn_rl_repo")

import ml_dtypes
import concourse.bass as bass
import concourse.bacc as bacc
import concourse.mybir as mybir
import concourse.tile as tile
from concourse.bass_utils import run_bass_kernel_spmd

T, B, L = 512, 64, 48
START, STOP = 46, 47
NCORES = 8
K = 64                      # scan steps per core (core 7: 63 real + 1 pad)
MU = 0.4287                 # mean per-step log-growth, keeps |log q| < ~5
P2 = 2 * L                  # 96 partitions: (batch parity, label)
NB2 = B // 2                # 32 batch pairs
SC = L * NB2                # 1536 state columns: (basis j, batch pair b2)

_FP = mybir.dt.float32
_BF = mybir.dt.bfloat16
_cache = {}


def _build(endings):
    """endings: per-core tuple of (slot, b2, k) extraction points."""
    nc = bacc.Bacc()
    dT = nc.declare_dram_parameter("dT", [P2, K * NB2], _FP, isOutput=False)
    w2 = nc.declare_dram_parameter("w2", [P2, P2], _BF, isOutput=False)
    sinit = nc.declare_dram_parameter("sinit", [P2, SC], _BF, isOutput=False)
    mout = nc.declare_dram_parameter("mout", [P2, SC], _BF, isOutput=True)
    sel = nc.declare_dram_parameter("sel", [P2, B * L], _BF, isOutput=True)

    by_k = [[] for _ in range(K)]
    for (slot, b2, k) in endings:
        by_k[k].append((slot, b2))

    with tile.TileContext(nc) as tc:
        with (
            tc.tile_pool(name="consts", bufs=1) as consts,
            tc.tile_pool(name="state", bufs=1) as state,
            tc.tile_pool(name="spool", bufs=3) as spool,
            tc.tile_pool(name="tpool", bufs=3) as tpool,
            tc.tile_pool(name="ps0", bufs=2, space="PSUM") as ps0,
            tc.tile_pool(name="ps1", bufs=2, space="PSUM") as ps1,
            tc.tile_pool(name="ps2", bufs=2, space="PSUM") as ps2,
        ):
            w2_sb = consts.tile([P2, P2], _BF)
            nc.gpsimd.dma_start(w2_sb[:], w2[:])
            s0 = state.tile([P2, SC], _BF)
            nc.gpsimd.dma_start(s0[:], sinit[:])
            dT_sb = state.tile([P2, K * NB2], _FP)
            for c in range(2):
                cs = slice(c * K * NB2 // 2, (c + 1) * K * NB2 // 2)
                nc.gpsimd.dma_start(dT_sb[:, cs], dT[:, cs])
            dh = state.tile([P2, K * NB2], _BF)
            for c in range(4):
                cs = slice(c * K * NB2 // 4, (c + 1) * K * NB2 // 4)
                nc.scalar.activation(dh[:, cs], dT_sb[:, cs],
                                     mybir.ActivationFunctionType.Exp)

            sel_sb = state.tile([P2, B * L], _BF)
            nc.gpsimd.memzero(sel_sb[:])

            s_prev = s0
            for k in range(K):
                s_new = spool.tile([P2, SC], _BF, tag="s")
                tmp = tpool.tile([P2, 1024], _BF, tag="tmp")
                dsl = dh[:, k * NB2:(k + 1) * NB2]
                p0 = ps0.tile([P2, 512], _FP, tag="p0")
                nc.tensor.matmul(p0[:], w2_sb[:], s_prev[:, 0:512])
                nc.scalar.copy(tmp[:, 0:512], p0[:])
                p1 = ps1.tile([P2, 512], _FP, tag="p1")
                nc.tensor.matmul(p1[:], w2_sb[:], s_prev[:, 512:1024])
                nc.scalar.copy(tmp[:, 512:1024], p1[:])
                p2 = ps2.tile([P2, 512], _FP, tag="p2")
                nc.tensor.matmul(p2[:], w2_sb[:], s_prev[:, 1024:1536])
                # D-multiply: cols 0:1024 read the bf16 SBUF bounce (4x DVE
                # mode), cols 1024:1536 read PSUM directly (1x).
                for h in range(2):
                    hs = slice(h * 512, (h + 1) * 512)
                    nc.vector.tensor_mul(
                        s_new[:, hs].rearrange('u (j b) -> u j b', j=16),
                        tmp[:, hs].rearrange('u (j b) -> u j b', j=16),
                        dsl.unsqueeze(1).broadcast_to([P2, 16, NB2]))
                nc.vector.tensor_mul(
                    s_new[:, 1024:1536].rearrange('u (j b) -> u j b', j=16),
                    p2[:].rearrange('u (j b) -> u j b', j=16),
                    dsl.unsqueeze(1).broadcast_to([P2, 16, NB2]))
                for (slot, b2) in by_k[k]:
                    src = s_new[:].rearrange('u (j b) -> u j b', j=L)
                    nc.gpsimd.tensor_copy(
                        sel_sb[:, slot * L:(slot + 1) * L],
                        src[:, :, b2:b2 + 1].squeeze(2))
                s_prev = s_new
            nc.gpsimd.dma_start(mout[:], s_prev[:])
            nc.gpsimd.dma_start(sel[:], sel_sb[:])
    nc.finalize()
    return nc


def _get_nc(endings_key, endings):
    if _cache.get("key") != endings_key:
        _cache["nc"] = _build(endings)
        _cache["key"] = endings_key
    return _cache["nc"]


def kernel(feats, transitions, tags, mask):
    feats = np.asarray(feats, np.float32)
    transitions = np.asarray(transitions, np.float32)
    tags_in = np.asarray(tags).astype(np.int64)
    mask_in = np.asarray(mask).astype(bool)

    tr64 = transitions.astype(np.float64)
    E = np.exp(tr64)                       # E[i,j] = exp(trans i->j)
    estop = np.exp(tr64[:, STOP])
    lengths = mask_in.sum(1).astype(np.int64)

    f64 = feats.astype(np.float64)
    m = f64.max(2)
    c = m + np.log(np.exp(f64 - m[:, :, None]).sum(2)) + MU      # (T, B)
    fhat = f64 - c[:, :, None]                                   # (T, B, L)
    Slog = np.cumsum(c, axis=0)
    q0 = np.exp(fhat[0]) * np.exp(tr64[START])[None, :]          # (B, L)

    # per-sequence end step -> (core, slot, b, local k)
    endings = [[] for _ in range(NCORES)]
    host_only = []                    # sequences ending at t=0 (never occurs
    for b in range(B):                # for the spec'd lengths, but be safe)
        tstar = int(lengths[b]) - 1
        if tstar == 0:
            host_only.append(b)
            continue
        cstar = (tstar - 1) // K
        kstar = tstar - K * cstar - 1
        endings[cstar].append((len(endings[cstar]), b, kstar))
    endings_key = tuple(lengths.tolist())
    # build program uses (slot, b2, k) per core; same program for all cores
    # would diverge, so the union of all cores' extraction points runs on
    # every core (harmless extra DMAs), with per-core slot collisions
    # avoided by a global slot numbering.
    all_pts = []
    for ci in range(NCORES):
        for (slot, b, kst) in endings[ci]:
            all_pts.append((ci, slot, b, kst))
    # global slots: one per sequence
    prog_pts = tuple((gslot, b // 2, kst)
                     for gslot, (ci, slot, b, kst) in enumerate(all_pts))
    nc = _get_nc(endings_key, prog_pts)

    w2 = np.zeros((P2, P2), np.float64)
    w2[0:L, 0:L] = E
    w2[L:P2, L:P2] = E
    w2_bf = w2.astype(ml_dtypes.bfloat16)
    sinit = np.zeros((P2, SC), np.float64)
    i_idx = np.arange(L)
    for p in range(2):
        sinit[p * L + i_idx[:, None], i_idx[:, None] * NB2 + np.arange(NB2)[None, :]] = 1.0
    sinit_bf = sinit.astype(ml_dtypes.bfloat16)

    in_maps = []
    for ci in range(NCORES):
        chunk = np.zeros((K, B, L), np.float64)
        lo = K * ci + 1
        hi = min(lo + K, T)
        chunk[:hi - lo] = fhat[lo:hi]
        # dT[p*48+i, k*32+b2] = chunk[k, 2*b2+p, i]
        dTc = np.ascontiguousarray(
            chunk.reshape(K, NB2, 2, L).transpose(2, 3, 0, 1).reshape(
                P2, K * NB2)).astype(np.float32)
        in_maps.append({
            "dT": dTc,
            "w2": w2_bf,
            "sinit": sinit_bf,
        })

    tmpbase = os.environ.get("BASS_KERNEL_TMPDIR")
    if tmpbase:
        import tempfile
        tmpbase = tempfile.mkdtemp(dir=tmpbase)
    bkr = run_bass_kernel_spmd(
        nc, in_maps, list(range(NCORES)), tmpdir=tmpbase)
    global LAST_EXEC_NS
    LAST_EXEC_NS = bkr.exec_time_ns
    res = bkr.results

    # ---- host combine: chunk boundary vectors (float64) ----
    qin = np.zeros((NCORES, B, L))
    qin[0] = q0
    for ci in range(NCORES - 1):
        M = np.asarray(res[ci]["mout"]).astype(np.float64).reshape(
            2, L, L, NB2)                     # (p, i', j, b2)
        qv = qin[ci].reshape(NB2, 2, L)       # (b2, p, j)
        qin[ci + 1] = np.einsum('pijb,bpj->bpi', M, qv).reshape(B, L)

    # ---- forward score from extracted end states ----
    fwd = 0.0
    for gslot, (ci, slot, b, kst) in enumerate(all_pts):
        selc = np.asarray(res[ci]["sel"]).astype(np.float64)
        Sb = selc[:, gslot * L:(gslot + 1) * L]       # (96, 48)
        p = b % 2
        Sjb = Sb[p * L:(p + 1) * L, :]                # (i', j)
        z = estop @ (Sjb @ qin[ci][b])
        tstar = int(lengths[b]) - 1
        fwd += np.log(z) + Slog[tstar, b]
    for b in host_only:
        fwd += np.log(estop @ q0[b]) + Slog[0, b]

    # ---- gold path score (pure gathers, float64) ----
    tagsT = tags_in.T
    prev = np.concatenate([np.full((1, B), START, np.int64), tagsT[:-1]], 0)
    emit = np.take_along_axis(f64, tagsT[:, :, None], 2)[..., 0]
    tg = np.where(mask_in.T, emit + tr64[prev, tagsT], 0.0).sum()
    end_ids = tagsT[lengths - 1, np.arange(B)]
    gold = tg + tr64[end_ids, STOP].sum()

    return np.float32(fwd - gold)
